# revision 22
# baseline (speedup 1.0000x reference)
"""Multi-head self-attention (RoPE, causal) on 8 Trainium2 NeuronCores.

Problem: B=1, S=2048, D=1024, H=16 heads, d_k=64, causal, interleaved RoPE.

Sharding: tensor-parallel over heads for QKV+attention (2 heads/core),
then AllToAll to switch to sequence sharding, so the output projection
is fully local (each core computes y rows [256c:256c+256] for all 1024
output dims). Host concatenates row slices — no host arithmetic.

v2 structure (vs v1): projections and attention are interleaved per
512-wide q-chunk j so ACT's exp and DVE/Pool softmax work overlap the
next chunk's PE matmuls; WO weights are SBUF-resident (loaded once in
the prologue); the softmax probabilities (pt), attention output
(attnT), and the AllToAll payload are fp16 (halves DVE mask cost, the
collective, and staging DMA); av_ps is read directly from PSUM for the
normalize (no avu copy); PSUM->SBUF copies ride gpsimd where DVE is
busy.

Device layouts (per core c, local heads h0=2c, h1=2c+1):
  xt   [1024,2048]  x^T (d on partitions) — replicated
  qt/kt [128,2048]  Q^T/K^T rows: [h0-even(32) h0-odd(32) h1-even h1-odd]
                    (RoPE pair-permutation folded into weight slices;
                     scores are invariant to a shared permutation of d_k)
  v_sb [128,16*65]  V' tiles [V(64) | ones] per k-tile (ones col -> softmax
                    sums ride the AV matmul as output row 64)
  scores S^T [k,q] in PSUM -> exp on ACT (fp16 out) -> P^T tiles ->
  AV: attn^T = V'^T P; normalize with 1/sums (partition_broadcast) into
  attnT [128,2048] fp16 (rows 0:64 head0, 64:128 head1)
  A2A shards attn^T -> each core gets attnT[:, S_c] for all 16 heads
  WO: y[s,m] accumulated over 8 e-tiles (weights resident f32r).

Matmuls run in float32r (TF32-like, full PE rate at N>=256); fp16
moving operands (AV, WO stationary side) run at full rate at any width.
"""

import math
import numpy as np

import concourse.bass as bass
import concourse.mybir as mybir
import concourse.tile as tile
from concourse import bacc
from concourse.bass_utils import run_bass_kernel_spmd

F32 = mybir.dt.float32
F32R = mybir.dt.float32r
FP16 = mybir.dt.float16
AF = mybir.ActivationFunctionType
ALU = mybir.AluOpType

S = 2048
D = 1024
H = 16
DK = 64
NCORES = 8
EC = D // NCORES          # 128 e-dims per core (2 heads)
SC = S // NCORES          # 256 s-rows per core after A2A
NQ = 512                  # q-chunk width
NJ = S // NQ              # 4 q-chunks
KT = S // 128             # 16 k-tiles
DT = D // 128             # 8 d-tiles
THETA = 10000.0

_PROGRAM = None

_HINTS = (mybir.EngineType.PE, mybir.EngineType.Activation,
          mybir.EngineType.DVE, mybir.EngineType.Pool,
          mybir.EngineType.SP)


def _build_program(reps=1, collective=True, loop_stages=("single",),
                   look=3, ptbufs=5, **_ignored):
    nc = bacc.Bacc("TRN2", target_bir_lowering=False, debug=False,
                   num_devices=NCORES if collective else 1)

    # ---- DRAM I/O ----
    xt_d = nc.dram_tensor("xt", [D, S], F32R, kind="ExternalInput").ap()
    wqt_d = nc.dram_tensor("wqt", [D, EC], F32R, kind="ExternalInput").ap()
    wkt_d = nc.dram_tensor("wkt", [D, EC], F32R, kind="ExternalInput").ap()
    wvt_d = nc.dram_tensor("wvt", [D, EC], F32R, kind="ExternalInput").ap()
    wot_d = nc.dram_tensor("wot", [D, D], FP16, kind="ExternalInput").ap()
    ident_d = nc.dram_tensor("ident", [128, 128], FP16,
                             kind="ExternalInput").ap()
    ctab_d = nc.dram_tensor("ctab", [128, S], F32, kind="ExternalInput").ap()
    stab_d = nc.dram_tensor("stab", [128, S], F32, kind="ExternalInput").ap()
    pswap_d = nc.dram_tensor("pswap", [128, 128], F32R,
                             kind="ExternalInput").ap()
    msk_d = nc.dram_tensor("msk01", [128, 128], FP16, kind="ExternalInput").ap()
    y_d = nc.dram_tensor("y_out", [SC, D], F32, kind="ExternalOutput").ap()

    # internal DRAM for the collective (fp16 payload)
    a2a_in = nc.dram_tensor("a2a_in", [NCORES, EC, SC], FP16)
    a2a_out = nc.dram_tensor("a2a_out", [NCORES, EC, SC], FP16)

    with tile.TileContext(nc) as tc:
        with (
            tc.tile_pool(name="persist", bufs=1) as pp,
            tc.tile_pool(name="work", bufs=3) as wp,
            tc.tile_pool(name="pt_pool", bufs=ptbufs) as ptp,
            tc.tile_pool(name="psum", bufs=2, space="PSUM") as ps,
            tc.tile_pool(name="psum_att", bufs=3, space="PSUM") as psa,
        ):
            # ---- resident loads (prologue, outside the timing loop) ----
            wqt = pp.tile([128, DT * EC], F32R)   # [d-tile part, t*EC+e]
            wkt = pp.tile([128, DT * EC], F32R)
            wvt = pp.tile([128, DT * EC], F32R)
            for t in range(DT):
                sl = slice(128 * t, 128 * (t + 1))
                nc.sync.dma_start(wqt[:, EC * t:EC * (t + 1)], wqt_d[sl, :])
                nc.sync.dma_start(wkt[:, EC * t:EC * (t + 1)], wkt_d[sl, :])
                nc.sync.dma_start(wvt[:, EC * t:EC * (t + 1)], wvt_d[sl, :])
            ctab = pp.tile([128, S], F32)
            stab = pp.tile([128, S], F32)
            pswap = pp.tile([128, 128], F32R)
            msk01 = pp.tile([128, 128], FP16)
            ident16 = pp.tile([128, 128], FP16)
            nc.sync.dma_start(pswap[:], pswap_d[:])
            nc.sync.dma_start(msk01[:], msk_d[:])
            nc.sync.dma_start(ident16[:], ident_d[:])
            # fp16: the AV matmul pairs fp16 P with fp16 V (walrus requires
            # matching dtypes when f32/f32r is involved, so f32r V would
            # force f32r P)
            v_sb = [pp.tile([128, KT * 65], FP16, name=f"v{h}")
                    for h in range(2)]
            for h in range(2):
                # softmax-sum ones column, written once (memset beats a
                # 4-byte-strided DMA by orders of magnitude here)
                nc.gpsimd.memset(v_sb[h][:, 64::65], 1.0)
            # x + trig j-major so chunk-j consumers start after ~1/NJ of
            # the x DMA instead of all of it.
            xt = [pp.tile([128, S], F32R, name=f"xt{t}") for t in range(DT)]
            for jc in range(NJ):
                csl_ = slice(NQ * jc, NQ * (jc + 1))
                nc.sync.dma_start(ctab[:, csl_], ctab_d[:, csl_])
                nc.sync.dma_start(stab[:, csl_], stab_d[:, csl_])
                for t in range(DT):
                    nc.sync.dma_start(
                        xt[t][:, csl_],
                        xt_d[128 * t:128 * (t + 1), csl_])
            # WO weights resident (fp16, host-converted), loaded AFTER x so
            # they don't delay the first projection chunk.
            wot = pp.tile([128, DT * 1024], FP16)  # [e-tile part, t*1024+m]
            for t in range(DT):
                nc.sync.dma_start(wot[:, 1024 * t:1024 * (t + 1)],
                                  wot_d[128 * t:128 * (t + 1), :])

            qt = pp.tile([128, S], F32R)   # RoPE'd Q^T
            kt = pp.tile([128, S], F32R)   # RoPE'd K^T
            attnT = pp.tile([128, S], FP16)     # rows 64h..64h+64 = head h
            at_all = pp.tile([128, DT * SC], FP16)  # post-A2A [e, s_c]

            def proj_chunk(j):
                # Q/K projections + RoPE for q-chunk j; V for k-tiles
                # 4j..4j+3.  Consumes only chunk-j x DMAs.
                csl = slice(NQ * j, NQ * (j + 1))
                for (wt, out_sb) in ((wqt, qt), (wkt, kt)):
                    g_ps = ps.tile([128, NQ], F32, name="g_ps", tag="mm")
                    for t in range(DT):
                        nc.tensor.matmul(
                            g_ps[:],
                            wt[:, EC * t:EC * (t + 1)],
                            xt[t][:, csl],
                            start=(t == 0), stop=(t == DT - 1))
                    # RoPE: rot = g*ctab + swap(g)*stab
                    graw = wp.tile([128, NQ], F32R, name="graw")
                    nc.vector.tensor_copy(graw[:], g_ps[:])
                    gsw_ps = ps.tile([128, NQ], F32, name="gsw_ps", tag="mm")
                    nc.tensor.matmul(gsw_ps[:], pswap[:], graw[:],
                                     start=True, stop=True)
                    a_sb = wp.tile([128, NQ], F32, name="a_sb")
                    nc.gpsimd.tensor_mul(a_sb[:], graw[:].bitcast(F32),
                                         ctab[:, csl])
                    b_sb = wp.tile([128, NQ], F32, name="b_sb")
                    nc.vector.tensor_mul(b_sb[:], gsw_ps[:], stab[:, csl])
                    nc.vector.tensor_add(out_sb[:, csl], a_sb[:], b_sb[:])
                # V via V^T (x moving, 512-wide, no duplication) then PE
                # transpose back to [s, v] for the AV stationary layout:
                # 8x512 + 4x128 PE cycles vs 32x256 for the dup-V scheme.
                vt_ps = ps.tile([128, NQ], F32, name="vt_ps", tag="mm")
                for t in range(DT):
                    nc.tensor.matmul(
                        vt_ps[:],
                        wvt[:, EC * t:EC * (t + 1)],
                        xt[t][:, csl],
                        start=(t == 0), stop=(t == DT - 1))
                vt16 = wp.tile([128, NQ], FP16, name="vt16")
                nc.vector.tensor_copy(vt16[:], vt_ps[:])
                tp_ps = ps.tile([128, NQ], FP16, name="tp_ps", tag="mm")
                for q in range(4):
                    nc.tensor.transpose(tp_ps[:, 128 * q:128 * (q + 1)],
                                        vt16[:, 128 * q:128 * (q + 1)],
                                        ident16[:])
                for q in range(4):
                    st = 4 * j + q
                    for h in range(2):
                        nc.vector.tensor_copy(
                            v_sb[h][:, 65 * st:65 * st + 64],
                            tp_ps[:, 128 * q + 64 * h:128 * q + 64 * (h + 1)])

            def att_chunk(j):
                # k-tiles in pairs: two score MMs (start/stop=True each, own
                # bank halves of one [128, <=1024] st2 span) -> one exp per
                # pair (fp16 out). Diagonal causal masks = DVE 0/1 multiplies
                # on the pt slices. AV issues LOOK pairs behind the scores so
                # PE never waits on ACT.
                scale = 1.0 / math.sqrt(DK)
                av_ps = [psa.tile([65, NQ], F32, name=f"av_ps{h}",
                                  tag="av", bufs=2) for h in range(2)]
                ndiag = 4 * j
                npairs = (ndiag + 4) // 2
                pairs = [(p, h) for p in range(npairs) for h in range(2)]
                pend = {}

                def emit_score(k):
                    p, h = pairs[k]
                    i0 = 2 * p
                    rs = [i0 - ndiag, i0 + 1 - ndiag]
                    offs = [128 * r if r > 0 else 0 for r in rs]
                    ws = [NQ - o for o in offs]
                    cs = [0, ws[0]]
                    wtot = ws[0] + ws[1]
                    hs = slice(64 * h, 64 * (h + 1))
                    st2 = psa.tile([128, 2 * NQ], F32, name=f"st2{h}",
                                   tag="st2", bufs=2)
                    for q in range(2):
                        nc.tensor.matmul(
                            st2[:, cs[q]:cs[q] + ws[q]],
                            kt[hs, 128 * (i0 + q):128 * (i0 + q + 1)],
                            qt[hs, NQ * j + offs[q]:NQ * (j + 1)],
                            start=True, stop=True,
                            tile_position=(64 * h, 0))
                    pt = ptp.tile([128, 2 * NQ], FP16, name="pt")
                    nc.scalar.activation(pt[:, :wtot], st2[:, :wtot],
                                         AF.Exp, scale=scale)
                    if rs[0] >= 0:
                        for q in range(2):
                            nc.vector.tensor_mul(
                                pt[:, cs[q]:cs[q] + 128],
                                pt[:, cs[q]:cs[q] + 128],
                                msk01[:])
                    pend[k] = (pt, i0, offs, ws, cs)

                def emit_av(k):
                    p, h = pairs[k]
                    pt, i0, offs, ws, cs = pend.pop(k)
                    for q in range(2):
                        ii = i0 + q
                        nc.tensor.matmul(
                            av_ps[h][:, offs[q]:],
                            v_sb[h][:, 65 * ii:65 * (ii + 1)],
                            pt[:, cs[q]:cs[q] + ws[q]],
                            start=(ii == 0), stop=(ii == ndiag + 3))

                n = len(pairs)
                for k in range(n + look):
                    if k < n:
                        emit_score(k)
                    if k >= look:
                        emit_av(k - look)
                return av_ps

            def norm_stage_chunk(j, av_ps):
                # normalize straight out of PSUM: rec=1/sums (row 64),
                # broadcast over the 64 v-dims, multiply into attnT fp16.
                # Emitted one chunk late so it overlaps att(j+1) on PE.
                for h in range(2):
                    rec = wp.tile([1, NQ], F32, name="rec")
                    nc.vector.reciprocal(rec[:], av_ps[h][64:65, :])
                    bc = wp.tile([64, NQ], F32, name="bc", tag="b_sb")
                    nc.gpsimd.partition_broadcast(bc[:], rec[:])
                    nc.vector.tensor_mul(
                        attnT[64 * h:64 * (h + 1), NQ * j:NQ * (j + 1)],
                        av_ps[h][0:64, :], bc[:])
                # A2A staging: dest cores 2j, 2j+1 own q-cols inside chunk j
                for r in (2 * j, 2 * j + 1):
                    nc.sync.dma_start(
                        a2a_in.ap()[r, :, :],
                        attnT[:, SC * r:SC * (r + 1)])

            def body(interleave=True):
                av_prev = None
                if interleave:
                    for j in range(NJ):
                        proj_chunk(j)
                        if av_prev is not None:
                            norm_stage_chunk(j - 1, av_prev)
                        av_prev = att_chunk(j)
                else:
                    for j in range(NJ):
                        proj_chunk(j)
                    for j in range(NJ):
                        if av_prev is not None:
                            norm_stage_chunk(j - 1, av_prev)
                        av_prev = att_chunk(j)
                norm_stage_chunk(NJ - 1, av_prev)

            def att_all():
                av_prev = None
                for j in range(NJ):
                    if av_prev is not None:
                        norm_stage_chunk(j - 1, av_prev)
                    av_prev = att_chunk(j)
                norm_stage_chunk(NJ - 1, av_prev)

            def wo_stage():
                src = a2a_out if collective else a2a_in
                for t in range(DT):
                    nc.sync.dma_start(at_all[:, SC * t:SC * (t + 1)],
                                      src.ap()[t, :, :])
                for n in range(2):          # m-chunks of 512
                    # y_ps shares the "av" PSUM buffers (free by WO time;
                    # keeps the bank budget at 8)
                    y_ps = [psa.tile([128, 512], F32, name=f"y_ps{sub}",
                                     tag="av", bufs=2) for sub in range(SC // 128)]
                    for t in range(DT):
                        for sub in range(SC // 128):
                            nc.tensor.matmul(
                                y_ps[sub][:],
                                at_all[:, SC * t + 128 * sub:
                                       SC * t + 128 * (sub + 1)],
                                wot[:, 1024 * t + 512 * n:
                                    1024 * t + 512 * (n + 1)],
                                start=(t == 0), stop=(t == DT - 1))
                    for sub in range(SC // 128):
                        y_sb = wp.tile([128, 512], F32, name="y_sb")
                        nc.vector.tensor_copy(y_sb[:], y_ps[sub][:])
                        nc.sync.dma_start(
                            y_d[128 * sub:128 * (sub + 1),
                                512 * n:512 * (n + 1)], y_sb[:])

            if reps > 1:
                # per-stage loop modes for HW stage timing; "single" is the
                # full pass (what test.py measures)
                if loop_stages == ("single",):
                    with tc.For_i(0, reps, 1, hint_engines=_HINTS):
                        body()
                        wo_stage()
                elif loop_stages == ("seq",):
                    with tc.For_i(0, reps, 1, hint_engines=_HINTS):
                        body(interleave=False)
                        wo_stage()
                elif loop_stages == ("proj",):
                    with tc.For_i(0, reps, 1, hint_engines=_HINTS):
                        for j in range(NJ):
                            proj_chunk(j)
                elif loop_stages == ("att",):
                    for j in range(NJ):
                        proj_chunk(j)
                    with tc.For_i(0, reps, 1, hint_engines=_HINTS):
                        att_all()
                elif loop_stages == ("wo",):
                    body()
                    with tc.For_i(0, reps, 1, hint_engines=_HINTS):
                        wo_stage()
                else:
                    raise ValueError(f"unknown loop_stages {loop_stages}")
            else:
                body()
            if collective:
                nc.gpsimd.collective_compute(
                    "AllToAll", ALU.bypass,
                    replica_groups=[list(range(NCORES))],
                    ins=[a2a_in.ap().opt()],
                    outs=[a2a_out.ap().opt()],
                )
            if reps == 1:
                wo_stage()

    nc.compile()
    return nc


def _get_program():
    global _PROGRAM
    if _PROGRAM is None:
        _PROGRAM = _build_program()
    return _PROGRAM


def _host_prep(x, token_positions, WQ, WK, WV, WO):
    x = np.asarray(x, dtype=np.float32)
    WQ = np.asarray(WQ, dtype=np.float32)
    WK = np.asarray(WK, dtype=np.float32)
    WV = np.asarray(WV, dtype=np.float32)
    WO = np.asarray(WO, dtype=np.float32)
    pos = np.asarray(token_positions).reshape(-1).astype(np.float32)

    xt = np.ascontiguousarray(x.reshape(S, D).T)            # [D, S]

    inv_freq = (1.0 / (THETA ** (np.arange(0, DK, 2, dtype=np.float32)
                                 / np.float32(DK)))).astype(np.float32)
    ang = pos[:, None] * inv_freq[None, :]                  # [S, 32] f32
    cos = np.cos(ang).astype(np.float32).T                  # [32, S]
    sin = np.sin(ang).astype(np.float32).T
    ctab = np.ascontiguousarray(np.tile(cos, (4, 1)))       # [128, S]
    stab = np.ascontiguousarray(
        np.concatenate([-sin, sin, -sin, sin], axis=0))     # [128, S]

    pswap = np.zeros((128, 128), np.float32)
    for i in range(128):
        blk, o = divmod(i, 32)
        j = (blk ^ 1) * 32 + o
        pswap[j, i] = 1.0

    msk01 = (np.arange(128)[None, :] >= np.arange(128)[:, None]) \
        .astype(np.float16)                         # keep f >= p

    perm = np.concatenate([np.arange(0, DK, 2), np.arange(1, DK, 2)])
    in_maps = []
    for c in range(NCORES):
        rows = np.concatenate([128 * c + 64 * l + perm for l in range(2)])
        wqt = np.ascontiguousarray(WQ[rows, :].T)           # [D, EC]
        wkt = np.ascontiguousarray(WK[rows, :].T)
        vrows = np.arange(128 * c, 128 * (c + 1))
        wvt = np.ascontiguousarray(WV[vrows, :].T)          # [D, EC]
        in_maps.append({
            "xt": xt, "wqt": wqt, "wkt": wkt, "wvt": wvt,
            "wot": np.ascontiguousarray(WO.T).astype(np.float16),
            "ctab": ctab, "stab": stab, "pswap": pswap,
            "msk01": msk01,
            "ident": np.eye(128, dtype=np.float16),
        })
    return in_maps


def kernel(x, token_positions, WQ, WK, WV, WO):
    in_maps = _host_prep(x, token_positions, WQ, WK, WV, WO)
    nc = _get_program()
    res = run_bass_kernel_spmd(nc, in_maps, list(range(NCORES)))
    y = np.concatenate([res.results[c]["y_out"] for c in range(NCORES)],
                       axis=0)
    return y.reshape(1, S, D).astype(np.float32)


# revision 32
# speedup vs baseline: 1.0705x; 1.0705x over previous
"""Multi-head self-attention (RoPE, causal) on 8 Trainium2 NeuronCores.

Problem: B=1, S=2048, D=1024, H=16 heads, d_k=64, causal, interleaved RoPE.

Sharding: tensor-parallel over heads for QKV+attention (2 heads/core),
then AllToAll to switch to sequence sharding, so the output projection
is fully local (each core computes y rows [256c:256c+256] for all 1024
output dims). Host concatenates row slices — no host arithmetic.

v2 structure (vs v1): projections and attention are interleaved per
512-wide q-chunk j so ACT's exp and DVE/Pool softmax work overlap the
next chunk's PE matmuls; WO weights are SBUF-resident (loaded once in
the prologue); the softmax probabilities (pt), attention output
(attnT), and the AllToAll payload are fp16 (halves DVE mask cost, the
collective, and staging DMA); av_ps is read directly from PSUM for the
normalize (no avu copy); PSUM->SBUF copies ride gpsimd where DVE is
busy.

Device layouts (per core c, local heads h0=2c, h1=2c+1):
  xt   [1024,2048]  x^T (d on partitions) — replicated
  qt/kt [128,2048]  Q^T/K^T rows: [h0-even(32) h0-odd(32) h1-even h1-odd]
                    (RoPE pair-permutation folded into weight slices;
                     scores are invariant to a shared permutation of d_k)
  v_sb [128,16*65]  V' tiles [V(64) | ones] per k-tile (ones col -> softmax
                    sums ride the AV matmul as output row 64)
  scores S^T [k,q] in PSUM -> exp on ACT (fp16 out) -> P^T tiles ->
  AV: attn^T = V'^T P; normalize with 1/sums (partition_broadcast) into
  attnT [128,2048] fp16 (rows 0:64 head0, 64:128 head1)
  A2A shards attn^T -> each core gets attnT[:, S_c] for all 16 heads
  WO: y[s,m] accumulated over 8 e-tiles (weights resident f32r).

Matmuls run in float32r (TF32-like, full PE rate at N>=256); fp16
moving operands (AV, WO stationary side) run at full rate at any width.
"""

import math
import numpy as np

import concourse.bass as bass
import concourse.mybir as mybir
import concourse.tile as tile
from concourse import bacc
from concourse.bass_utils import run_bass_kernel_spmd

F32 = mybir.dt.float32
F32R = mybir.dt.float32r
FP16 = mybir.dt.float16
AF = mybir.ActivationFunctionType
ALU = mybir.AluOpType

S = 2048
D = 1024
H = 16
DK = 64
NCORES = 8
EC = D // NCORES          # 128 e-dims per core (2 heads)
SC = S // NCORES          # 256 s-rows per core after A2A
NQ = 512                  # q-chunk width
NJ = S // NQ              # 4 q-chunks
KT = S // 128             # 16 k-tiles
DT = D // 128             # 8 d-tiles
THETA = 10000.0

_PROGRAM = None

_HINTS = (mybir.EngineType.PE, mybir.EngineType.Activation,
          mybir.EngineType.DVE, mybir.EngineType.Pool,
          mybir.EngineType.SP)


def _build_program(reps=1, collective=True, loop_stages=("single",),
                   look=3, ptbufs=5, **_ignored):
    nc = bacc.Bacc("TRN2", target_bir_lowering=False, debug=False,
                   num_devices=NCORES if collective else 1)

    # ---- DRAM I/O ----
    xt_d = nc.dram_tensor("xt", [D, S], F32R, kind="ExternalInput").ap()
    wqt_d = nc.dram_tensor("wqt", [D, EC], F32R, kind="ExternalInput").ap()
    wkt_d = nc.dram_tensor("wkt", [D, EC], F32R, kind="ExternalInput").ap()
    wvt2_d = nc.dram_tensor("wvt2", [D, 256], F32R, kind="ExternalInput").ap()
    wot_d = nc.dram_tensor("wot", [D, D], FP16, kind="ExternalInput").ap()
    ctab_d = nc.dram_tensor("ctab", [128, S], F32, kind="ExternalInput").ap()
    stab_d = nc.dram_tensor("stab", [128, S], F32, kind="ExternalInput").ap()
    pswap_d = nc.dram_tensor("pswap", [128, 128], F32R,
                             kind="ExternalInput").ap()
    msk_d = nc.dram_tensor("msk01", [128, 128], FP16, kind="ExternalInput").ap()
    y_d = nc.dram_tensor("y_out", [SC, D], F32, kind="ExternalOutput").ap()

    # internal DRAM for the collective (fp16 payload)
    a2a_in = nc.dram_tensor("a2a_in", [NCORES, EC, SC], FP16)
    a2a_out = nc.dram_tensor("a2a_out", [NCORES, EC, SC], FP16)

    with tile.TileContext(nc) as tc:
        with (
            tc.tile_pool(name="persist", bufs=1) as pp,
            tc.tile_pool(name="work", bufs=3) as wp,
            tc.tile_pool(name="pt_pool", bufs=ptbufs) as ptp,
            tc.tile_pool(name="psum", bufs=2, space="PSUM") as ps,
            tc.tile_pool(name="psum_att", bufs=3, space="PSUM") as psa,
        ):
            # ---- resident loads (prologue, outside the timing loop) ----
            wqt = pp.tile([128, DT * EC], F32R)   # [d-tile part, t*EC+e]
            wkt = pp.tile([128, DT * EC], F32R)
            wvt2 = pp.tile([128, DT * 256], F32R)
            for t in range(DT):
                sl = slice(128 * t, 128 * (t + 1))
                nc.sync.dma_start(wqt[:, EC * t:EC * (t + 1)], wqt_d[sl, :])
                nc.sync.dma_start(wkt[:, EC * t:EC * (t + 1)], wkt_d[sl, :])
                nc.sync.dma_start(wvt2[:, 256 * t:256 * (t + 1)], wvt2_d[sl, :])
            ctab = pp.tile([128, S], F32)
            stab = pp.tile([128, S], F32)
            pswap = pp.tile([128, 128], F32R)
            msk01 = pp.tile([128, 128], FP16)
            nc.sync.dma_start(pswap[:], pswap_d[:])
            nc.sync.dma_start(msk01[:], msk_d[:])
            # fp16: the AV matmul pairs fp16 P with fp16 V (walrus requires
            # matching dtypes when f32/f32r is involved, so f32r V would
            # force f32r P)
            v_sb = [pp.tile([128, KT * 65], FP16, name=f"v{h}")
                    for h in range(2)]
            for h in range(2):
                # softmax-sum ones column, written once (memset beats a
                # 4-byte-strided DMA by orders of magnitude here)
                nc.gpsimd.memset(v_sb[h][:, 64::65], 1.0)
            # x + trig j-major so chunk-j consumers start after ~1/NJ of
            # the x DMA instead of all of it.
            xt = [pp.tile([128, S], F32R, name=f"xt{t}") for t in range(DT)]
            for jc in range(NJ):
                csl_ = slice(NQ * jc, NQ * (jc + 1))
                nc.sync.dma_start(ctab[:, csl_], ctab_d[:, csl_])
                nc.sync.dma_start(stab[:, csl_], stab_d[:, csl_])
                for t in range(DT):
                    nc.sync.dma_start(
                        xt[t][:, csl_],
                        xt_d[128 * t:128 * (t + 1), csl_])
            # WO weights resident (fp16, host-converted), loaded AFTER x so
            # they don't delay the first projection chunk.
            wot = pp.tile([128, DT * 1024], FP16)  # [e-tile part, t*1024+m]
            for t in range(DT):
                nc.sync.dma_start(wot[:, 1024 * t:1024 * (t + 1)],
                                  wot_d[128 * t:128 * (t + 1), :])

            qt = pp.tile([128, S], F32R)   # RoPE'd Q^T
            kt = pp.tile([128, S], F32R)   # RoPE'd K^T
            attnT = pp.tile([128, S], FP16)     # rows 64h..64h+64 = head h
            at_all = pp.tile([128, DT * SC], FP16)  # post-A2A [e, s_c]

            def proj_chunk(j):
                # Q/K projections + RoPE for q-chunk j; V for k-tiles
                # 4j..4j+3.  Consumes only chunk-j x DMAs.
                csl = slice(NQ * j, NQ * (j + 1))
                for (wt, out_sb) in ((wqt, qt), (wkt, kt)):
                    g_ps = ps.tile([128, NQ], F32, name="g_ps", tag="mm")
                    for t in range(DT):
                        nc.tensor.matmul(
                            g_ps[:],
                            wt[:, EC * t:EC * (t + 1)],
                            xt[t][:, csl],
                            start=(t == 0), stop=(t == DT - 1))
                    # RoPE: rot = g*ctab + swap(g)*stab
                    graw = wp.tile([128, NQ], F32R, name="graw")
                    nc.vector.tensor_copy(graw[:], g_ps[:])
                    gsw_ps = ps.tile([128, NQ], F32, name="gsw_ps", tag="mm")
                    nc.tensor.matmul(gsw_ps[:], pswap[:], graw[:],
                                     start=True, stop=True)
                    a_sb = wp.tile([128, NQ], F32, name="a_sb")
                    nc.gpsimd.tensor_mul(a_sb[:], graw[:].bitcast(F32),
                                         ctab[:, csl])
                    b_sb = wp.tile([128, NQ], F32, name="b_sb")
                    nc.vector.tensor_mul(b_sb[:], gsw_ps[:], stab[:, csl])
                    nc.vector.tensor_add(out_sb[:, csl], a_sb[:], b_sb[:])
                for st in range(4 * j, 4 * (j + 1)):
                    v_ps = ps.tile([128, NQ], F32, name="v_ps", tag="mm")
                    for t in range(DT):
                        nc.tensor.matmul(
                            v_ps[:, :256],
                            xt[t][:, 128 * st:128 * (st + 1)],
                            wvt2[:, 256 * t:256 * (t + 1)],
                            start=(t == 0), stop=(t == DT - 1))
                    for h in range(2):
                        nc.vector.tensor_copy(
                            v_sb[h][:, 65 * st:65 * st + 64],
                            v_ps[:, 64 * h:64 * (h + 1)])

            def att_chunk(j):
                # k-tiles in pairs: two score MMs (start/stop=True each, own
                # bank halves of one [128, <=1024] st2 span) -> one exp per
                # pair (fp16 out). Diagonal causal masks = DVE 0/1 multiplies
                # on the pt slices. AV issues LOOK pairs behind the scores so
                # PE never waits on ACT.
                scale = 1.0 / math.sqrt(DK)
                av_ps = [psa.tile([65, NQ], F32, name=f"av_ps{h}",
                                  tag="av", bufs=2) for h in range(2)]
                ndiag = 4 * j
                npairs = (ndiag + 4) // 2
                pairs = [(p, h) for p in range(npairs) for h in range(2)]
                pend = {}

                def emit_score(k):
                    p, h = pairs[k]
                    i0 = 2 * p
                    rs = [i0 - ndiag, i0 + 1 - ndiag]
                    offs = [128 * r if r > 0 else 0 for r in rs]
                    ws = [NQ - o for o in offs]
                    cs = [0, ws[0]]
                    wtot = ws[0] + ws[1]
                    hs = slice(64 * h, 64 * (h + 1))
                    st2 = psa.tile([128, 2 * NQ], F32, name=f"st2{h}",
                                   tag="st2", bufs=2)
                    for q in range(2):
                        nc.tensor.matmul(
                            st2[:, cs[q]:cs[q] + ws[q]],
                            kt[hs, 128 * (i0 + q):128 * (i0 + q + 1)],
                            qt[hs, NQ * j + offs[q]:NQ * (j + 1)],
                            start=True, stop=True,
                            tile_position=(64 * h, 0))
                    pt = ptp.tile([128, 2 * NQ], FP16, name="pt")
                    nc.scalar.activation(pt[:, :wtot], st2[:, :wtot],
                                         AF.Exp, scale=scale)
                    if rs[0] >= 0:
                        for q in range(2):
                            nc.vector.tensor_mul(
                                pt[:, cs[q]:cs[q] + 128],
                                pt[:, cs[q]:cs[q] + 128],
                                msk01[:])
                    pend[k] = (pt, i0, offs, ws, cs)

                def emit_av(k):
                    p, h = pairs[k]
                    pt, i0, offs, ws, cs = pend.pop(k)
                    for q in range(2):
                        ii = i0 + q
                        nc.tensor.matmul(
                            av_ps[h][:, offs[q]:],
                            v_sb[h][:, 65 * ii:65 * (ii + 1)],
                            pt[:, cs[q]:cs[q] + ws[q]],
                            start=(ii == 0), stop=(ii == ndiag + 3))

                n = len(pairs)
                for k in range(n + look):
                    if k < n:
                        emit_score(k)
                    if k >= look:
                        emit_av(k - look)
                return av_ps

            def norm_stage_chunk(j, av_ps):
                # normalize straight out of PSUM: rec=1/sums (row 64),
                # broadcast over the 64 v-dims, multiply into attnT fp16.
                # Emitted one chunk late so it overlaps att(j+1) on PE.
                for h in range(2):
                    rec = wp.tile([1, NQ], F32, name="rec")
                    nc.vector.reciprocal(rec[:], av_ps[h][64:65, :])
                    bc = wp.tile([64, NQ], F32, name="bc", tag="b_sb")
                    nc.gpsimd.partition_broadcast(bc[:], rec[:])
                    nc.vector.tensor_mul(
                        attnT[64 * h:64 * (h + 1), NQ * j:NQ * (j + 1)],
                        av_ps[h][0:64, :], bc[:])
                # A2A staging: dest cores 2j, 2j+1 own q-cols inside chunk j
                for r in (2 * j, 2 * j + 1):
                    nc.sync.dma_start(
                        a2a_in.ap()[r, :, :],
                        attnT[:, SC * r:SC * (r + 1)])

            def body(interleave=True):
                av_prev = None
                if interleave:
                    for j in range(NJ):
                        proj_chunk(j)
                        if av_prev is not None:
                            norm_stage_chunk(j - 1, av_prev)
                        av_prev = att_chunk(j)
                else:
                    for j in range(NJ):
                        proj_chunk(j)
                    for j in range(NJ):
                        if av_prev is not None:
                            norm_stage_chunk(j - 1, av_prev)
                        av_prev = att_chunk(j)
                norm_stage_chunk(NJ - 1, av_prev)

            def att_all():
                av_prev = None
                for j in range(NJ):
                    if av_prev is not None:
                        norm_stage_chunk(j - 1, av_prev)
                    av_prev = att_chunk(j)
                norm_stage_chunk(NJ - 1, av_prev)

            def wo_stage():
                src = a2a_out if collective else a2a_in
                for t in range(DT):
                    nc.sync.dma_start(at_all[:, SC * t:SC * (t + 1)],
                                      src.ap()[t, :, :])
                for n in range(2):          # m-chunks of 512
                    # y_ps shares the "av" PSUM buffers (free by WO time;
                    # keeps the bank budget at 8)
                    y_ps = [psa.tile([128, 512], F32, name=f"y_ps{sub}",
                                     tag="av", bufs=2) for sub in range(SC // 128)]
                    for t in range(DT):
                        for sub in range(SC // 128):
                            nc.tensor.matmul(
                                y_ps[sub][:],
                                at_all[:, SC * t + 128 * sub:
                                       SC * t + 128 * (sub + 1)],
                                wot[:, 1024 * t + 512 * n:
                                    1024 * t + 512 * (n + 1)],
                                start=(t == 0), stop=(t == DT - 1))
                    for sub in range(SC // 128):
                        y_sb = wp.tile([128, 512], F32, name="y_sb")
                        nc.vector.tensor_copy(y_sb[:], y_ps[sub][:])
                        nc.sync.dma_start(
                            y_d[128 * sub:128 * (sub + 1),
                                512 * n:512 * (n + 1)], y_sb[:])

            if reps > 1:
                # per-stage loop modes for HW stage timing; "single" is the
                # full pass (what test.py measures)
                if loop_stages == ("single",):
                    with tc.For_i(0, reps, 1, hint_engines=_HINTS):
                        body()
                        wo_stage()
                elif loop_stages == ("seq",):
                    with tc.For_i(0, reps, 1, hint_engines=_HINTS):
                        body(interleave=False)
                        wo_stage()
                elif loop_stages == ("proj",):
                    with tc.For_i(0, reps, 1, hint_engines=_HINTS):
                        for j in range(NJ):
                            proj_chunk(j)
                elif loop_stages == ("att",):
                    for j in range(NJ):
                        proj_chunk(j)
                    with tc.For_i(0, reps, 1, hint_engines=_HINTS):
                        att_all()
                elif loop_stages == ("wo",):
                    body()
                    with tc.For_i(0, reps, 1, hint_engines=_HINTS):
                        wo_stage()
                else:
                    raise ValueError(f"unknown loop_stages {loop_stages}")
            else:
                body()
            if collective:
                nc.gpsimd.collective_compute(
                    "AllToAll", ALU.bypass,
                    replica_groups=[list(range(NCORES))],
                    ins=[a2a_in.ap().opt()],
                    outs=[a2a_out.ap().opt()],
                )
            if reps == 1:
                wo_stage()

    nc.compile()
    return nc


def _get_program():
    global _PROGRAM
    if _PROGRAM is None:
        _PROGRAM = _build_program()
    return _PROGRAM


def _host_prep(x, token_positions, WQ, WK, WV, WO):
    x = np.asarray(x, dtype=np.float32)
    WQ = np.asarray(WQ, dtype=np.float32)
    WK = np.asarray(WK, dtype=np.float32)
    WV = np.asarray(WV, dtype=np.float32)
    WO = np.asarray(WO, dtype=np.float32)
    pos = np.asarray(token_positions).reshape(-1).astype(np.float32)

    xt = np.ascontiguousarray(x.reshape(S, D).T)            # [D, S]

    inv_freq = (1.0 / (THETA ** (np.arange(0, DK, 2, dtype=np.float32)
                                 / np.float32(DK)))).astype(np.float32)
    ang = pos[:, None] * inv_freq[None, :]                  # [S, 32] f32
    cos = np.cos(ang).astype(np.float32).T                  # [32, S]
    sin = np.sin(ang).astype(np.float32).T
    ctab = np.ascontiguousarray(np.tile(cos, (4, 1)))       # [128, S]
    stab = np.ascontiguousarray(
        np.concatenate([-sin, sin, -sin, sin], axis=0))     # [128, S]

    pswap = np.zeros((128, 128), np.float32)
    for i in range(128):
        blk, o = divmod(i, 32)
        j = (blk ^ 1) * 32 + o
        pswap[j, i] = 1.0

    msk01 = (np.arange(128)[None, :] >= np.arange(128)[:, None]) \
        .astype(np.float16)                         # keep f >= p

    perm = np.concatenate([np.arange(0, DK, 2), np.arange(1, DK, 2)])
    in_maps = []
    for c in range(NCORES):
        rows = np.concatenate([128 * c + 64 * l + perm for l in range(2)])
        wqt = np.ascontiguousarray(WQ[rows, :].T)           # [D, EC]
        wkt = np.ascontiguousarray(WK[rows, :].T)
        vrows = np.arange(128 * c, 128 * (c + 1))
        wvt = WV[vrows, :].T                                # [D, EC]
        wvt2 = np.ascontiguousarray(np.concatenate([wvt, wvt], axis=1))
        in_maps.append({
            "xt": xt, "wqt": wqt, "wkt": wkt, "wvt2": wvt2,
            "wot": np.ascontiguousarray(WO.T).astype(np.float16),
            "ctab": ctab, "stab": stab, "pswap": pswap,
            "msk01": msk01,
        })
    return in_maps


def kernel(x, token_positions, WQ, WK, WV, WO):
    in_maps = _host_prep(x, token_positions, WQ, WK, WV, WO)
    nc = _get_program()
    res = run_bass_kernel_spmd(nc, in_maps, list(range(NCORES)))
    y = np.concatenate([res.results[c]["y_out"] for c in range(NCORES)],
                       axis=0)
    return y.reshape(1, S, D).astype(np.float32)
